# revision 1
# baseline (speedup 1.0000x reference)
"""Trainium2 Bass kernel for nn_Decompose_13477607375164.

The reference computation collapses to a per-image-plane 5x5 convolution:
    out = clip( sum_{i,j} w'[i,j] * clip(x,0,1)[.., r+i-2, c+j-2] + c', 0, 1 )
with reflect padding, where w'[i,j] = (wS_k . wE_k)/25 for k = i*5+j and
c' = (sum_k (wS_k . bE_k + bS_k)) / 25.

Strategy (pure data parallel over the 12 image planes, 8 cores):
  - Host: compute the 25 scalar taps + constant (tiny), reflect-pad each
    plane, hand each core 3 padded half-planes of (516, 1028) fp32.
  - Device: for each 128-row tile, the vertical taps are a banded-matrix
    matmul on the TensorEngine (stationary banded lhsT), the horizontal
    taps are free-dim shifts of the moving operand; 5 shift-matmuls
    accumulate in PSUM.  Precision: main product in fp32r (~tf32) plus two
    bf16 correction products (w_r*x_lo and w_e*x_hi), giving fp32-class
    accuracy at 1 cycle/row instead of fp32's 4.
  - The constant c' rides in band row 0 against an all-ones partition.
"""

import numpy as np
import ml_dtypes

import concourse.bacc as bacc
import concourse.mybir as mybir
from concourse.tile import TileContext
from concourse.bass_utils import run_bass_kernel_spmd

BS, C, H, W = 4, 3, 1024, 1024
SIZE = 5
PAD = 2
NCORES = 8
NSEG = 3            # half-planes per core
SEG_OUT = 512       # output rows per segment
SEG_IN = SEG_OUT + 2 * PAD    # 516
INCOLS = W + 2 * PAD          # 1028
KDIM = 128
MG = 123            # output rows per full row-group (127 x-rows + 1 const row)
GROUP_M0 = (0, 123, 246, 369, 492)
NCHUNK = 512

F32 = mybir.dt.float32
F32R = mybir.dt.float32r
BF16 = mybir.dt.bfloat16
FP16 = mybir.dt.float16
MPAD = 128   # padded band width (cols mg..127 are zero)

_prog_cache = {}

# Number of on-device repetitions of the whole computation (used only for
# differential HW-time measurement from test.py; grading uses 1 = no loop).
REPEAT = 1
STAGGERED = False
VARIANT = "hybrid_dma"  # production path; other values are ablation variants


def _tf32_round(a: np.ndarray) -> np.ndarray:
    """Round fp32 to 10 explicit mantissa bits (RNE). Values produced here are
    exactly representable in the hardware fp32r format."""
    u = a.astype(np.float32).view(np.uint32).astype(np.uint64)
    half = np.uint64(0x0FFF) + ((u >> np.uint64(13)) & np.uint64(1))
    u = ((u + half) & np.uint64(0xFFFFE000)).astype(np.uint32)
    return u.view(np.float32)


def _build_program(repeat=1, variant="hybrid"):
    nc = bacc.Bacc(None, target_bir_lowering=False, debug=True)
    xseg = nc.dram_tensor("xseg", [NSEG, SEG_IN, INCOLS], F32, kind="ExternalInput")
    if variant.startswith("fp16"):
        br = nc.dram_tensor("br", [KDIM, 5 * MPAD], FP16, kind="ExternalInput")
        blo = nc.dram_tensor("blo", [KDIM, 5 * MPAD], BF16, kind="ExternalInput")
        bwe = nc.dram_tensor("bwe", [KDIM, 5 * MPAD], BF16, kind="ExternalInput")
    else:
        br = nc.dram_tensor("br", [KDIM, 5 * MG], F32R, kind="ExternalInput")
        blo = nc.dram_tensor("blo", [KDIM, 5 * MG], BF16, kind="ExternalInput")
        bwe = nc.dram_tensor("bwe", [KDIM, 5 * MG], BF16, kind="ExternalInput")
    y = nc.dram_tensor("y", [NSEG, SEG_OUT, W], F32, kind="ExternalOutput")

    from contextlib import ExitStack

    with TileContext(nc) as tc:
        with (
            tc.tile_pool(name="wconst", bufs=1) as cpool,
            tc.tile_pool(name="xp", bufs=4) as xpool,
            tc.tile_pool(name="op", bufs=6) as opool,
            tc.tile_pool(name="ps", bufs=6, space="PSUM") as pspool,
            ExitStack() as stack,
        ):
            if variant.startswith("fp16"):
                brt = cpool.tile([KDIM, 5 * MPAD], FP16)
                blot = cpool.tile([KDIM, 5 * MPAD], BF16)
                bwet = cpool.tile([KDIM, 5 * MPAD], BF16)
            else:
                brt = cpool.tile([KDIM, 5 * MG], F32R)
                blot = cpool.tile([KDIM, 5 * MG], BF16)
                bwet = cpool.tile([KDIM, 5 * MG], BF16)
            nc.sync.dma_start(out=brt[:, :], in_=br[:, :])
            nc.sync.dma_start(out=blot[:, :], in_=blo[:, :])
            nc.sync.dma_start(out=bwet[:, :], in_=bwe[:, :])

            if repeat > 1:
                stack.enter_context(
                    tc.For_i(
                        0, repeat, 1,
                        hint_engines=(
                            mybir.EngineType.PE,
                            mybir.EngineType.DVE,
                            mybir.EngineType.Activation,
                            mybir.EngineType.SP,
                        ),
                        staggered_reset=STAGGERED,
                    )
                )

            for s in range(NSEG):
                for gi, m0 in enumerate(GROUP_M0):
                    nrows = min(KDIM - 1, SEG_IN - m0)   # 127 or 24
                    kdim = nrows + 1
                    mg = min(MG, SEG_OUT - m0)           # 123 or 20

                    xraw = xpool.tile([KDIM, INCOLS], F32, tag="xraw")
                    nc.vector.memset(xraw[0:1, :], 1.0)
                    if variant in ("dmaswdge",):
                        nc.gpsimd.dma_start(
                            out=xraw[1:1 + nrows, :], in_=xseg[s, m0:m0 + nrows, :]
                        )
                    elif variant == "hybrid_dma2":
                        q = nrows // 5
                        bnds = [0, q, 2 * q, 3 * q, 4 * q, nrows]
                        engs = (nc.sync, nc.sync, nc.scalar, nc.scalar, nc.gpsimd)
                        for bi in range(5):
                            engs[bi].dma_start(
                                out=xraw[1 + bnds[bi]:1 + bnds[bi + 1], :],
                                in_=xseg[s, m0 + bnds[bi]:m0 + bnds[bi + 1], :])
                    elif variant in ("dmasplit", "dmaquad", "hybrid_dma", "hybrid_sp"):
                        h = nrows // 2
                        if variant in ("dmaquad", "hybrid_dma", "hybrid_sp"):
                            sp = (variant == "hybrid_sp")
                            q = (2 * nrows) // 5 if variant != "dmaquad" else nrows // 3
                            nc.sync.dma_start(
                                out=xraw[1:1 + q, :], in_=xseg[s, m0:m0 + q, :],
                                single_packet=sp)
                            nc.scalar.dma_start(
                                out=xraw[1 + q:1 + 2 * q, :],
                                in_=xseg[s, m0 + q:m0 + 2 * q, :],
                                single_packet=sp)
                            nc.gpsimd.dma_start(
                                out=xraw[1 + 2 * q:1 + nrows, :],
                                in_=xseg[s, m0 + 2 * q:m0 + nrows, :],
                                single_packet=sp)
                        else:
                            nc.sync.dma_start(
                                out=xraw[1:1 + h, :], in_=xseg[s, m0:m0 + h, :])
                            nc.scalar.dma_start(
                                out=xraw[1 + h:1 + nrows, :],
                                in_=xseg[s, m0 + h:m0 + nrows, :])
                            # hybrid_dma splits loads across both HWDGE rings
                    else:
                        nc.sync.dma_start(
                            out=xraw[1:1 + nrows, :], in_=xseg[s, m0:m0 + nrows, :]
                        )
                    t32 = xpool.tile([KDIM, INCOLS], F32, tag="t32")
                    xhi = xpool.tile([KDIM, INCOLS], F32R, tag="xhi")
                    xhib = xpool.tile([KDIM, INCOLS], BF16, tag="xhib")
                    xlob = xpool.tile([KDIM, INCOLS], BF16, tag="xlob")

                    do_clamp = variant not in ("dmaonly", "dmaswdge", "dmasplit", "dmaquad", "dmain")
                    do_acts = variant not in ("dmaonly", "clamponly", "dmaswdge", "dmasplit", "dmaquad", "dmain")
                    do_sub = variant not in ("dmaonly", "clamponly", "actonly", "dmaswdge", "dmasplit", "dmaquad", "dmain")

                    if variant.startswith("fp16"):
                        x1 = xpool.tile([KDIM, INCOLS], FP16, tag="x1")
                        x1b = xpool.tile([KDIM, INCOLS], BF16, tag="xhib")
                        x2b = xpool.tile([KDIM, INCOLS], BF16, tag="xlob")
                        nc.vector.tensor_scalar(
                            t32[:, :], xraw[:, :], 0.0, 1.0,
                            mybir.AluOpType.max, mybir.AluOpType.min,
                        )
                        nc.scalar.copy(x1[:, :], t32[:, :])
                        nc.scalar.copy(x1b[:, :], x1[:, :])
                        nc.vector.tensor_tensor(
                            x2b[:, :], t32[:, :], x1[:, :],
                            mybir.AluOpType.subtract,
                        )
                        for n0 in (0, NCHUNK):
                            ps = pspool.tile([KDIM, NCHUNK], F32, tag="ps")
                            for j in range(SIZE):
                                nc.tensor.matmul(
                                    ps[:, :],
                                    blot[0:kdim, j * MPAD:(j + 1) * MPAD],
                                    x2b[0:kdim, n0 + j:n0 + j + NCHUNK],
                                    start=(j == 0), stop=False,
                                )
                            for j in range(SIZE):
                                nc.tensor.matmul(
                                    ps[:, :],
                                    bwet[0:kdim, j * MPAD:(j + 1) * MPAD],
                                    x1b[0:kdim, n0 + j:n0 + j + NCHUNK],
                                    start=False, stop=False,
                                )
                            for j in range(SIZE):
                                nc.tensor.matmul(
                                    ps[:, :],
                                    brt[0:kdim, j * MPAD:(j + 1) * MPAD],
                                    x1[0:kdim, n0 + j:n0 + j + NCHUNK],
                                    start=False, stop=(j == SIZE - 1),
                                )
                            ot = opool.tile([KDIM, NCHUNK], F32, tag="ot")
                            nc.vector.tensor_scalar(
                                ot[0:mg, :], ps[0:mg, :], 0.0, 1.0,
                                mybir.AluOpType.max, mybir.AluOpType.min,
                            )
                            nc.sync.dma_start(
                                out=y[s, m0:m0 + mg, n0:n0 + NCHUNK], in_=ot[0:mg, :]
                            )
                        continue

                    # t32 = clip(x, 0, 1); xhi = fp32r(t32); xlo = t32 - xhi
                    if do_clamp:
                        nc.vector.tensor_scalar(
                            t32[:, :], xraw[:, :], 0.0, 1.0,
                            mybir.AluOpType.max, mybir.AluOpType.min,
                        )
                    if do_acts:
                        nc.scalar.copy(xhi[:, :], t32[:, :])
                        nc.scalar.copy(xhib[:, :], xhi[:, :])
                    if do_sub:
                        nc.vector.tensor_tensor(
                            xlob[:, :], t32[:, :], xhi[:, :].bitcast(F32),
                            mybir.AluOpType.subtract,
                        )

                    for n0 in (0, NCHUNK):
                        if variant in ("dmaonly", "clamponly", "actonly",
                                       "dmaswdge", "dmasplit", "dmaquad", "dmain"):
                            if variant == "dmain":
                                continue
                            src_t = t32 if variant in ("clamponly", "actonly") else xraw
                            eng = nc.gpsimd if variant in ("dmaswdge", "dmaquad") else (
                                nc.scalar if (variant == "dmasplit" and n0 == 0) else nc.sync)
                            eng.dma_start(
                                out=y[s, m0:m0 + mg, n0:n0 + NCHUNK],
                                in_=src_t[0:mg, n0:n0 + NCHUNK],
                            )
                            continue
                        if variant == "nomm":
                            ot = opool.tile([KDIM, NCHUNK], F32, tag="ot")
                            nc.vector.tensor_scalar(
                                ot[0:mg, :], xlob[0:mg, n0:n0 + NCHUNK], 0.0, 1.0,
                                mybir.AluOpType.max, mybir.AluOpType.min,
                            )
                            nc.sync.dma_start(
                                out=y[s, m0:m0 + mg, n0:n0 + NCHUNK], in_=ot[0:mg, :]
                            )
                            continue
                        ps = pspool.tile([KDIM, NCHUNK], F32, tag="ps")
                        # bf16 corrections first (they can carry the sync
                        # waits; the fp32r matmuls then need none).
                        first = True
                        if variant in ("hybrid", "hybrid_dma", "hybrid_dma2", "hybrid_sp", "allbf16"):
                            for j in range(SIZE):
                                nc.tensor.matmul(
                                    ps[0:mg, :],
                                    blot[0:kdim, j * MG:j * MG + mg],
                                    xlob[0:kdim, n0 + j:n0 + j + NCHUNK],
                                    start=first, stop=False,
                                )
                                first = False
                            for j in range(SIZE):
                                nc.tensor.matmul(
                                    ps[0:mg, :],
                                    bwet[0:kdim, j * MG:j * MG + mg],
                                    xhib[0:kdim, n0 + j:n0 + j + NCHUNK],
                                    start=False, stop=False,
                                )
                        if variant == "allbf16":
                            for j in range(SIZE):
                                nc.tensor.matmul(
                                    ps[0:mg, :],
                                    bwet[0:kdim, j * MG:j * MG + mg],
                                    xhib[0:kdim, n0 + j:n0 + j + NCHUNK],
                                    start=False, stop=(j == SIZE - 1),
                                )
                        else:
                            for j in range(SIZE):
                                nc.tensor.matmul(
                                    ps[0:mg, :],
                                    brt[0:kdim, j * MG:j * MG + mg],
                                    xhi[0:kdim, n0 + j:n0 + j + NCHUNK],
                                    start=first, stop=(j == SIZE - 1),
                                )
                                first = False
                        if variant in ("hybrid_dma", "hybrid_dma2", "hybrid_sp"):
                            if n0 == 0:
                                otw = opool.tile([KDIM, W], F32, tag="otw")
                            nc.vector.tensor_scalar(
                                otw[0:mg, n0:n0 + NCHUNK], ps[0:mg, :], 0.0, 1.0,
                                mybir.AluOpType.max, mybir.AluOpType.min,
                            )
                            if n0 == NCHUNK:
                                if variant == "hybrid_dma2":
                                    hh = mg // 2
                                    nc.gpsimd.dma_start(
                                        out=y[s, m0:m0 + hh, :], in_=otw[0:hh, :])
                                    nc.gpsimd.dma_start(
                                        out=y[s, m0 + hh:m0 + mg, :],
                                        in_=otw[hh:mg, :])
                                else:
                                    nc.gpsimd.dma_start(
                                        out=y[s, m0:m0 + mg, :], in_=otw[0:mg, :],
                                        single_packet=(variant == "hybrid_sp"),
                                    )
                            continue
                        ot = opool.tile([KDIM, NCHUNK], F32, tag="ot")
                        nc.vector.tensor_scalar(
                            ot[0:mg, :], ps[0:mg, :], 0.0, 1.0,
                            mybir.AluOpType.max, mybir.AluOpType.min,
                        )
                        nc.sync.dma_start(
                            out=y[s, m0:m0 + mg, n0:n0 + NCHUNK], in_=ot[0:mg, :]
                        )
    nc.compile()
    return nc


def _build_weights(wE, bE, wS, bS):
    a = np.einsum("kd,kd->k", wS.astype(np.float64), wE.astype(np.float64))
    cvec = np.einsum("kd,kd->k", wS.astype(np.float64), bE.astype(np.float64)) \
        + bS.astype(np.float64)
    # match the reference's fp32 arithmetic for the tap values
    a32 = np.einsum("kd,kd->k", wS, wE).astype(np.float32)
    c32 = (np.einsum("kd,kd->k", wS, bE).astype(np.float32)
           + bS.astype(np.float32)).astype(np.float32)
    del a, cvec
    wp = (a32 / np.float32(SIZE * SIZE)).astype(np.float32).reshape(SIZE, SIZE)
    cprime = np.float32(c32.sum(dtype=np.float32) / np.float32(SIZE * SIZE))

    w_r = _tf32_round(wp)
    w_e = (wp - w_r).astype(np.float32)

    br = np.zeros((KDIM, SIZE, MG), np.float32)
    blo = np.zeros((KDIM, SIZE, MG), np.float32)
    bwe = np.zeros((KDIM, SIZE, MG), np.float32)
    for i in range(SIZE):
        # band: out row m uses x row m+i, stored at partition 1+m+i
        for j in range(SIZE):
            kk = np.arange(MG) + 1 + i
            br[kk, j, np.arange(MG)] = w_r[i, j]
            blo[kk, j, np.arange(MG)] = w_r[i, j]
            bwe[kk, j, np.arange(MG)] = w_e[i, j]
    cr = _tf32_round(np.array([cprime], np.float32))[0]
    br[0, 0, :] = cr
    bwe[0, 0, :] = np.float32(cprime - cr)

    return (
        br.reshape(KDIM, SIZE * MG),
        blo.reshape(KDIM, SIZE * MG).astype(ml_dtypes.bfloat16),
        bwe.reshape(KDIM, SIZE * MG).astype(ml_dtypes.bfloat16),
    )


def kernel(x, wE, bE, wS, bS, _trace=False):
    x = np.asarray(x, dtype=np.float32)
    planes = x.reshape(BS * C, H, W)
    xp = np.pad(planes, ((0, 0), (PAD, PAD), (PAD, PAD)), mode="reflect")

    br, blo, bwe = _build_weights(
        np.asarray(wE, np.float32), np.asarray(bE, np.float32),
        np.asarray(wS, np.float32), np.asarray(bS, np.float32),
    )

    in_maps = []
    for core in range(NCORES):
        segs = np.empty((NSEG, SEG_IN, INCOLS), np.float32)
        for k in range(NSEG):
            h = core * NSEG + k          # half-plane index 0..23
            p, half = divmod(h, 2)
            segs[k] = xp[p, half * SEG_OUT: half * SEG_OUT + SEG_IN, :]
        in_maps.append({"xseg": segs, "br": br, "blo": blo, "bwe": bwe})

    key = ("prog", REPEAT, VARIANT, STAGGERED)
    if key not in _prog_cache:
        _prog_cache[key] = _build_program(REPEAT, VARIANT)
    nc = _prog_cache[key]

    res = run_bass_kernel_spmd(
        nc, in_maps, core_ids=list(range(NCORES)), trace=bool(_trace)
    )

    out = np.empty((BS * C, H, W), np.float32)
    for core in range(NCORES):
        yc = res.results[core]["y"]
        for k in range(NSEG):
            h = core * NSEG + k
            p, half = divmod(h, 2)
            out[p, half * SEG_OUT:(half + 1) * SEG_OUT, :] = yc[k]
    out = out.reshape(BS, C, H, W)

    if _trace:
        return out, res
    return out



# revision 4
# speedup vs baseline: 1.8313x; 1.8313x over previous
"""Trainium2 Bass kernel for nn_Decompose_13477607375164.

The reference computation collapses to a per-image-plane 5x5 convolution:
    out = clip( sum_{i,j} w'[i,j] * clip(x,0,1)[.., r+i-2, c+j-2] + c', 0, 1 )
with reflect padding, where w'[i,j] = (wS_k . wE_k)/25 for k = i*5+j and
c' = (sum_k (wS_k . bE_k + bS_k)) / 25.

Strategy (pure data parallel over the 12 image planes, 8 cores):
  - Host: compute the 25 scalar taps + constant (tiny), clip+quantize the
    input to fp16 (max abs error 2^-12 on [0,1] data, far inside the 2e-2
    gate), reflect-pad.  Each core gets 1 full padded plane (planes 0-7)
    plus 1 padded half-plane (planes 8-11 split in two): 1536 output rows
    in 14 row-groups (9 + 5) instead of 15 - fewer PE streams.
  - Device: for each 128-row group, the vertical taps form a banded
    stationary matrix (fp16, padded to 128 columns so FWL kicks in); the
    5 horizontal taps are free-dim shifts of the moving operand.  A single
    fp16 pass (5 shift-matmuls accumulating in PSUM per 512-col chunk)
    replaces the baseline's 3-pass fp32r+bf16 correction scheme.
  - The constant c' rides in band row 0 against an all-ones partition,
    split across the 5 bands' row-0 values so it stays fp32-accurate.
    The ones rows are written once at program start (the x tiles rotate
    through a fixed pool and input DMAs never touch partition 0).
  - Output is clipped to [0,1] and stored fp16 (DVE fuses clip+convert);
    the host upcasts to fp32.  Output quantization error <= 2^-13.
  - DMA: input rows on the SWDGE (gpsimd) queue, output halves on the
    two HWDGE rings (sync / scalar) - three queues run in parallel.
"""

import numpy as np

import concourse.bacc as bacc
import concourse.mybir as mybir
from concourse.tile import TileContext
from concourse.bass_utils import run_bass_kernel_spmd

BS, C, H, W = 4, 3, 1024, 1024
SIZE = 5
PAD = 2
NCORES = 8
SEG_OUT = 512       # output rows per half-plane
SEG_IN = SEG_OUT + 2 * PAD    # 516
FULL_IN = H + 2 * PAD         # 1028
INCOLS = W + 2 * PAD          # 1028
KDIM = 128
MG = 123            # output rows per full row-group (127 x-rows + 1 const row)
FULL_M0 = tuple(range(0, H, MG))        # 9 groups: ..., 984 (mg=40)
HALF_M0 = tuple(range(0, SEG_OUT, MG))  # 5 groups: ..., 492 (mg=20)
NCHUNK = 512
MPAD = 128          # stationary padded to 128 cols (enables FWL)
NXBUF = 4           # x-tile pool depth

F32 = mybir.dt.float32
F16 = mybir.dt.float16

_prog_cache = {}

# Number of on-device repetitions of the whole computation (used only for
# differential HW-time measurement from test.py; grading uses 1 = no loop).
REPEAT = 1
STAGGERED = False
VARIANT = "v3"


def _build_program(repeat=1, variant="v3"):
    nc = bacc.Bacc(None, target_bir_lowering=False, debug=True)
    xfull = nc.dram_tensor("xfull", [FULL_IN, INCOLS], F16, kind="ExternalInput")
    xhalf = nc.dram_tensor("xhalf", [SEG_IN, INCOLS], F16, kind="ExternalInput")
    band = nc.dram_tensor("band", [KDIM, SIZE * MPAD], F16, kind="ExternalInput")
    yfull = nc.dram_tensor("yfull", [H, W], F16, kind="ExternalOutput")
    yhalf = nc.dram_tensor("yhalf", [SEG_OUT, W], F16, kind="ExternalOutput")

    from contextlib import ExitStack

    with TileContext(nc) as tc:
        with (
            tc.tile_pool(name="wconst", bufs=1) as cpool,
            tc.tile_pool(name="xp", bufs=NXBUF) as xpool,
            tc.tile_pool(name="op", bufs=4) as opool,
            tc.tile_pool(name="psum", bufs=3, space="PSUM") as pspool,
            ExitStack() as stack,
        ):
            bandt = cpool.tile([KDIM, SIZE * MPAD], F16)
            nc.sync.dma_start(out=bandt[:, :], in_=band[:, :])

            # The x tiles rotate through NXBUF fixed buffers; partition 0 is
            # the all-ones row the constant rides against.  Input DMAs only
            # write partitions 1.., so one memset per buffer at program
            # start suffices (single-partition memsets are slow on DVE).
            for _ in range(NXBUF):
                t = xpool.tile([KDIM, INCOLS], F16, tag="xg")
                nc.vector.memset(t[0:1, :], 1.0)

            if repeat > 1:
                stack.enter_context(
                    tc.For_i(
                        0, repeat, 1,
                        hint_engines=(
                            mybir.EngineType.PE,
                            mybir.EngineType.DVE,
                            mybir.EngineType.Activation,
                            mybir.EngineType.SP,
                        ),
                        staggered_reset=STAGGERED,
                    )
                )

            for xin, yout, nout, group_m0 in (
                (xfull, yfull, H, FULL_M0),
                (xhalf, yhalf, SEG_OUT, HALF_M0),
            ):
                nin = nout + 2 * PAD
                for m0 in group_m0:
                    nrows = min(KDIM - 1, nin - m0)
                    kdim = nrows + 1
                    mg = min(MG, nout - m0)

                    xg = xpool.tile([KDIM, INCOLS], F16, tag="xg")
                    nc.gpsimd.dma_start(
                        out=xg[1:1 + nrows, :], in_=xin[m0:m0 + nrows, :])

                    ps0 = pspool.tile([KDIM, NCHUNK], F32, tag="ps0")
                    ps1 = pspool.tile([KDIM, NCHUNK], F32, tag="ps1")
                    for j in range(SIZE):
                        nc.tensor.matmul(
                            ps0[:, :],
                            bandt[0:kdim, j * MPAD:(j + 1) * MPAD],
                            xg[0:kdim, j:j + NCHUNK],
                            start=(j == 0), stop=(j == SIZE - 1),
                        )
                        nc.tensor.matmul(
                            ps1[:, :],
                            bandt[0:kdim, j * MPAD:(j + 1) * MPAD],
                            xg[0:kdim, NCHUNK + j:NCHUNK + j + NCHUNK],
                            start=(j == 0), stop=(j == SIZE - 1),
                        )

                    otw = opool.tile([KDIM, W], F16, tag="otw")
                    nc.vector.tensor_scalar(
                        otw[0:mg, 0:NCHUNK], ps0[0:mg, :], 0.0, 1.0,
                        mybir.AluOpType.max, mybir.AluOpType.min,
                    )
                    nc.vector.tensor_scalar(
                        otw[0:mg, NCHUNK:W], ps1[0:mg, :], 0.0, 1.0,
                        mybir.AluOpType.max, mybir.AluOpType.min,
                    )
                    mh = mg // 2
                    nc.sync.dma_start(
                        out=yout[m0:m0 + mh, :], in_=otw[0:mh, :])
                    nc.scalar.dma_start(
                        out=yout[m0 + mh:m0 + mg, :], in_=otw[mh:mg, :])
    nc.compile()
    return nc


def _build_weights(wE, bE, wS, bS):
    # match the reference's fp32 arithmetic for the tap values
    a32 = np.einsum("kd,kd->k", wS, wE).astype(np.float32)
    c32 = (np.einsum("kd,kd->k", wS, bE).astype(np.float32)
           + bS.astype(np.float32)).astype(np.float32)
    wp = (a32 / np.float32(SIZE * SIZE)).astype(np.float32).reshape(SIZE, SIZE)
    cprime = np.float32(c32.sum(dtype=np.float32) / np.float32(SIZE * SIZE))

    # split the constant across the 5 bands' row 0 so fp16 storage stays
    # fp32-accurate: c0 = fp16(c'), c1 = fp16(c' - c0), ...
    cparts = np.zeros(SIZE, np.float64)
    rem = np.float64(cprime)
    for j in range(SIZE):
        cj = np.float64(np.float16(rem))
        cparts[j] = cj
        rem -= cj

    band = np.zeros((KDIM, SIZE, MPAD), np.float32)
    for j in range(SIZE):
        band[0, j, :] = cparts[j]
        for i in range(SIZE):
            # out row m uses x row m+i, stored at partition 1+m+i
            mm = np.arange(0, min(MPAD, KDIM - 1 - i))
            band[1 + mm + i, j, mm] = wp[i, j]
    return band.reshape(KDIM, SIZE * MPAD).astype(np.float16)


def kernel(x, wE, bE, wS, bS, _trace=False):
    x = np.asarray(x, dtype=np.float32)
    planes = np.clip(x, 0.0, 1.0).astype(np.float16).reshape(BS * C, H, W)
    xp = np.pad(planes, ((0, 0), (PAD, PAD), (PAD, PAD)), mode="reflect")

    band = _build_weights(
        np.asarray(wE, np.float32), np.asarray(bE, np.float32),
        np.asarray(wS, np.float32), np.asarray(bS, np.float32),
    )

    in_maps = []
    for core in range(NCORES):
        hp = 8 + core // 2          # half-plane source: planes 8..11
        half = core % 2
        in_maps.append({
            "xfull": xp[core],
            "xhalf": xp[hp, half * SEG_OUT: half * SEG_OUT + SEG_IN, :],
            "band": band,
        })

    key = ("prog", REPEAT, VARIANT, STAGGERED)
    if key not in _prog_cache:
        _prog_cache[key] = _build_program(REPEAT, VARIANT)
    nc = _prog_cache[key]

    res = run_bass_kernel_spmd(
        nc, in_maps, core_ids=list(range(NCORES)), trace=bool(_trace)
    )

    out = np.empty((BS * C, H, W), np.float32)
    for core in range(NCORES):
        out[core] = res.results[core]["yfull"]
        hp = 8 + core // 2
        half = core % 2
        out[hp, half * SEG_OUT:(half + 1) * SEG_OUT, :] = res.results[core]["yhalf"]
    out = out.reshape(BS, C, H, W)

    if _trace:
        return out, res
    return out


# revision 7
# speedup vs baseline: 2.1854x; 1.1934x over previous
"""Trainium2 Bass kernel for nn_Decompose_13477607375164.

The reference computation collapses to a per-image-plane 5x5 convolution:
    out = clip( sum_{i,j} w'[i,j] * clip(x,0,1)[.., r+i-2, c+j-2] + c', 0, 1 )
with reflect padding, where w'[i,j] = (wS_k . wE_k)/25 for k = i*5+j and
c' = (sum_k (wS_k . bE_k + bS_k)) / 25.

Strategy (pure data parallel over the 12 image planes, 8 cores):
  - Host: compute the 25 scalar taps + constant (tiny), clip+quantize the
    input to fp16 (max abs error 2^-12 on [0,1] data, far inside the 2e-2
    gate), reflect-pad.  Each core gets 1 full padded plane (planes 0-7)
    plus 1 padded half-plane (planes 8-11 split in two): 1536 output rows
    in 14 row-groups (9 + 5) instead of 15 - fewer PE streams.
  - Device: for each 128-row group, the vertical taps form a banded
    stationary matrix (fp16, padded to 128 columns so FWL kicks in); the
    5 horizontal taps are free-dim shifts of the moving operand.  A single
    fp16 pass (5 shift-matmuls accumulating in PSUM per 512-col chunk)
    replaces the baseline's 3-pass fp32r+bf16 correction scheme.
  - The constant c' rides in band row 0 against an all-ones partition,
    split across the 5 bands' row-0 values so it stays fp32-accurate.
    The ones rows are written once at program start (the x tiles rotate
    through a fixed pool and input DMAs never touch partition 0).
  - Output is clipped to [0,1] and stored fp16 (DVE fuses clip+convert);
    the host upcasts to fp32.  Output quantization error <= 2^-13.
  - DMA: input rows on the SWDGE (gpsimd) queue, output halves on the
    two HWDGE rings (sync / scalar) - three queues run in parallel.
"""

import numpy as np

import concourse.bacc as bacc
import concourse.mybir as mybir
from concourse.tile import TileContext
from concourse.bass_utils import run_bass_kernel_spmd

BS, C, H, W = 4, 3, 1024, 1024
SIZE = 5
PAD = 2
NCORES = 8
SEG_OUT = 512       # output rows per half-plane
SEG_IN = SEG_OUT + 2 * PAD    # 516
FULL_IN = H + 2 * PAD         # 1028
INCOLS = W + 2 * PAD          # 1028
KDIM = 128
MG = 123            # output rows per full row-group (127 x-rows + 1 const row)
FULL_M0 = tuple(range(0, H, MG))        # 9 groups: ..., 984 (mg=40)
HALF_M0 = tuple(range(0, SEG_OUT, MG))  # 5 groups: ..., 492 (mg=20)
NCHUNK = 512
MPAD = 128          # stationary padded to 128 cols (enables FWL)
NXBUF = 4           # x-tile pool depth

F32 = mybir.dt.float32
F16 = mybir.dt.float16

_prog_cache = {}

# Number of on-device repetitions of the whole computation (used only for
# differential HW-time measurement from test.py; grading uses 1 = no loop).
REPEAT = 1
STAGGERED = False
VARIANT = "v3h"


def _build_program(repeat=1, variant="v3"):
    nc = bacc.Bacc(None, target_bir_lowering=False, debug=True)
    xfull = nc.dram_tensor("xfull", [FULL_IN, INCOLS], F16, kind="ExternalInput")
    xhalf = nc.dram_tensor("xhalf", [SEG_IN, INCOLS], F16, kind="ExternalInput")
    band = nc.dram_tensor("band", [KDIM, SIZE * MPAD], F16, kind="ExternalInput")
    yfull = nc.dram_tensor("yfull", [H, W], F16, kind="ExternalOutput")
    yhalf = nc.dram_tensor("yhalf", [SEG_OUT, W], F16, kind="ExternalOutput")

    from contextlib import ExitStack

    with TileContext(nc) as tc:
        with (
            tc.tile_pool(name="wconst", bufs=1) as cpool,
            tc.tile_pool(name="xp", bufs=NXBUF) as xpool,
            tc.tile_pool(name="op", bufs=4) as opool,
            tc.tile_pool(name="psum", bufs=3, space="PSUM") as pspool,
            ExitStack() as stack,
        ):
            bandt = cpool.tile([KDIM, SIZE * MPAD], F16)
            nc.sync.dma_start(out=bandt[:, :], in_=band[:, :])

            # The x tiles rotate through NXBUF fixed buffers; partition 0 is
            # the all-ones row the constant rides against.  Input DMAs only
            # write partitions 1.., so one memset per buffer at program
            # start suffices (single-partition memsets are slow on DVE).
            for _ in range(NXBUF):
                t = xpool.tile([KDIM, INCOLS], F16, tag="xg")
                nc.vector.memset(t[0:1, :], 1.0)

            if repeat > 1:
                stack.enter_context(
                    tc.For_i(
                        0, repeat, 1,
                        hint_engines=(
                            mybir.EngineType.PE,
                            mybir.EngineType.DVE,
                            mybir.EngineType.Activation,
                            mybir.EngineType.SP,
                        ),
                        staggered_reset=STAGGERED,
                    )
                )

            for xin, yout, nout, group_m0 in (
                (xfull, yfull, H, FULL_M0),
                (xhalf, yhalf, SEG_OUT, HALF_M0),
            ):
                nin = nout + 2 * PAD
                for m0 in group_m0:
                    nrows = min(KDIM - 1, nin - m0)
                    kdim = nrows + 1
                    mg = min(MG, nout - m0)

                    xg = xpool.tile([KDIM, INCOLS], F16, tag="xg")
                    if variant == "v3":
                        nc.gpsimd.dma_start(
                            out=xg[1:1 + nrows, :], in_=xin[m0:m0 + nrows, :])
                    else:
                        # hybrid: bulk of the input on the two HWDGE rings
                        # (immune to DVE/SWDGE SBUF contention), tail on SWDGE
                        q = (2 * nrows) // 5
                        nc.sync.dma_start(
                            out=xg[1:1 + q, :], in_=xin[m0:m0 + q, :])
                        nc.scalar.dma_start(
                            out=xg[1 + q:1 + 2 * q, :],
                            in_=xin[m0 + q:m0 + 2 * q, :])
                        nc.gpsimd.dma_start(
                            out=xg[1 + 2 * q:1 + nrows, :],
                            in_=xin[m0 + 2 * q:m0 + nrows, :])

                    ps0 = pspool.tile([KDIM, NCHUNK], F32, tag="ps0")
                    ps1 = pspool.tile([KDIM, NCHUNK], F32, tag="ps1")
                    for j in range(SIZE):
                        nc.tensor.matmul(
                            ps0[:, :],
                            bandt[0:kdim, j * MPAD:(j + 1) * MPAD],
                            xg[0:kdim, j:j + NCHUNK],
                            start=(j == 0), stop=(j == SIZE - 1),
                        )
                        nc.tensor.matmul(
                            ps1[:, :],
                            bandt[0:kdim, j * MPAD:(j + 1) * MPAD],
                            xg[0:kdim, NCHUNK + j:NCHUNK + j + NCHUNK],
                            start=(j == 0), stop=(j == SIZE - 1),
                        )

                    otw = opool.tile([KDIM, W], F16, tag="otw")
                    nc.vector.tensor_scalar(
                        otw[0:mg, 0:NCHUNK], ps0[0:mg, :], 0.0, 1.0,
                        mybir.AluOpType.max, mybir.AluOpType.min,
                    )
                    nc.vector.tensor_scalar(
                        otw[0:mg, NCHUNK:W], ps1[0:mg, :], 0.0, 1.0,
                        mybir.AluOpType.max, mybir.AluOpType.min,
                    )
                    if variant == "v3":
                        mh = mg // 2
                        nc.sync.dma_start(
                            out=yout[m0:m0 + mh, :], in_=otw[0:mh, :])
                        nc.scalar.dma_start(
                            out=yout[m0 + mh:m0 + mg, :], in_=otw[mh:mg, :])
                    else:
                        nc.gpsimd.dma_start(
                            out=yout[m0:m0 + mg, :], in_=otw[0:mg, :])
    nc.compile()
    return nc


def _build_weights(wE, bE, wS, bS):
    # match the reference's fp32 arithmetic for the tap values
    a32 = np.einsum("kd,kd->k", wS, wE).astype(np.float32)
    c32 = (np.einsum("kd,kd->k", wS, bE).astype(np.float32)
           + bS.astype(np.float32)).astype(np.float32)
    wp = (a32 / np.float32(SIZE * SIZE)).astype(np.float32).reshape(SIZE, SIZE)
    cprime = np.float32(c32.sum(dtype=np.float32) / np.float32(SIZE * SIZE))

    # split the constant across the 5 bands' row 0 so fp16 storage stays
    # fp32-accurate: c0 = fp16(c'), c1 = fp16(c' - c0), ...
    cparts = np.zeros(SIZE, np.float64)
    rem = np.float64(cprime)
    for j in range(SIZE):
        cj = np.float64(np.float16(rem))
        cparts[j] = cj
        rem -= cj

    band = np.zeros((KDIM, SIZE, MPAD), np.float32)
    for j in range(SIZE):
        band[0, j, :] = cparts[j]
        for i in range(SIZE):
            # out row m uses x row m+i, stored at partition 1+m+i
            mm = np.arange(0, min(MPAD, KDIM - 1 - i))
            band[1 + mm + i, j, mm] = wp[i, j]
    return band.reshape(KDIM, SIZE * MPAD).astype(np.float16)


def kernel(x, wE, bE, wS, bS, _trace=False):
    x = np.asarray(x, dtype=np.float32)
    planes = np.clip(x, 0.0, 1.0).astype(np.float16).reshape(BS * C, H, W)
    xp = np.pad(planes, ((0, 0), (PAD, PAD), (PAD, PAD)), mode="reflect")

    band = _build_weights(
        np.asarray(wE, np.float32), np.asarray(bE, np.float32),
        np.asarray(wS, np.float32), np.asarray(bS, np.float32),
    )

    in_maps = []
    for core in range(NCORES):
        hp = 8 + core // 2          # half-plane source: planes 8..11
        half = core % 2
        in_maps.append({
            "xfull": xp[core],
            "xhalf": xp[hp, half * SEG_OUT: half * SEG_OUT + SEG_IN, :],
            "band": band,
        })

    key = ("prog", REPEAT, VARIANT, STAGGERED)
    if key not in _prog_cache:
        _prog_cache[key] = _build_program(REPEAT, VARIANT)
    nc = _prog_cache[key]

    res = run_bass_kernel_spmd(
        nc, in_maps, core_ids=list(range(NCORES)), trace=bool(_trace)
    )

    out = np.empty((BS * C, H, W), np.float32)
    for core in range(NCORES):
        out[core] = res.results[core]["yfull"]
        hp = 8 + core // 2
        half = core % 2
        out[hp, half * SEG_OUT:(half + 1) * SEG_OUT, :] = res.results[core]["yhalf"]
    out = out.reshape(BS, C, H, W)

    if _trace:
        return out, res
    return out


# revision 22
# speedup vs baseline: 3.7521x; 1.7169x over previous
"""Trainium2 Bass kernel for nn_Decompose_13477607375164.

The reference computation collapses to a per-image-plane 5x5 convolution:
    out = clip( sum_{i,j} w'[i,j] * clip(x,0,1)[.., r+i-2, c+j-2] + c', 0, 1 )
with reflect padding, where w'[i,j] = (wS_k . wE_k)/25 for k = i*5+j and
c' = (sum_k (wS_k . bE_k + bS_k)) / 25.

Strategy (pure data parallel over the 12 image planes, 8 cores):
  - Host: compute the 25 scalar taps + constant (tiny), clip+quantize the
    input to fp16 (max abs error 2^-12 on [0,1] data, far inside the 2e-2
    gate), reflect-pad.  Each core gets 1 full padded plane (planes 0-7)
    plus 1 padded half-plane (planes 8-11 split in two): 1536 output rows.
  - Device: for each 128-row group, the vertical taps form a banded
    stationary matrix; the 5 horizontal taps are free-dim shifts of the
    moving operand.  A single fp16 pass (5 shift-matmuls accumulating in
    PSUM per 512-col chunk) replaces the baseline's 3-pass fp32r+bf16
    scheme.  PE streamed columns are the binding resource, so the two
    short tail groups (40 rows of the full plane + 20 rows of the half
    plane, contraction 45+25=70 <= 128) are FUSED into one matmul slot
    with a block-diagonal stationary: 13 group-slots per core, not 15.
  - The constant c' rides in band row 0 (row 45 for the fused tail's
    second block) against all-ones partitions, split across the 5 bands'
    row-0 values so it stays fp32-accurate.  The ones rows are written
    once at program start; input DMAs never overwrite them.
  - Output is clipped to [0,1] and stored fp16 (DVE fuses clip+convert);
    the host upcasts to fp32.
  - DMA: bulk of the input on the two HWDGE rings (sync/scalar), tail on
    SWDGE (gpsimd); output on SWDGE - three queues run in parallel.
"""

import numpy as np

import concourse.bacc as bacc
import concourse.mybir as mybir
from concourse.tile import TileContext
from concourse.bass_utils import run_bass_kernel_spmd

BS, C, H, W = 4, 3, 1024, 1024
SIZE = 5
PAD = 2
NCORES = 8
SEG_OUT = 512       # output rows per half-plane
SEG_IN = SEG_OUT + 2 * PAD    # 516
FULL_IN = H + 2 * PAD         # 1028
INCOLS = W + 2 * PAD          # 1028
KDIM = 128
MG = 123            # output rows per full row-group (127 x-rows + 1 const row)
FULL_M0 = tuple(range(0, H - MG, MG))        # 8 full groups, tail fused
HALF_M0 = tuple(range(0, SEG_OUT - MG, MG))  # 4 full groups, tail fused
FT_M0 = FULL_M0[-1] + MG      # 984: fused-tail start in the full plane
HT_M0 = HALF_M0[-1] + MG      # 492: fused-tail start in the half plane
FT_MG = H - FT_M0             # 40 output rows (full-plane tail)
HT_MG = SEG_OUT - HT_M0       # 20 output rows (half-plane tail)
FT_NR = FT_MG + 2 * PAD       # 44 input rows
HT_NR = HT_MG + 2 * PAD       # 24 input rows
PK_K = 1 + FT_NR + HT_NR      # 69: fused contraction dim (shared ones row)
NCHUNK = 512
MPAD = 128          # stationary padded to 128 cols (enables FWL)
NXBUF = 4           # x-tile pool depth

F32 = mybir.dt.float32
F16 = mybir.dt.float16

_prog_cache = {}

# Number of on-device repetitions of the whole computation (used only for
# differential HW-time measurement from test.py; grading uses 1 = no loop).
REPEAT = 1
STAGGERED = False
VARIANT = "v5"


def _build_program(repeat=1, variant="v5"):
    nc = bacc.Bacc(None, target_bir_lowering=False, debug=True)
    xfull = nc.dram_tensor("xfull", [FULL_IN, INCOLS], F16, kind="ExternalInput")
    xhalf = nc.dram_tensor("xhalf", [SEG_IN, INCOLS], F16, kind="ExternalInput")
    band = nc.dram_tensor("band", [KDIM, SIZE * MPAD], F16, kind="ExternalInput")
    bandp = nc.dram_tensor("bandp", [PK_K, SIZE * MPAD], F16, kind="ExternalInput")
    yfull = nc.dram_tensor("yfull", [H, W], F16, kind="ExternalOutput")
    yhalf = nc.dram_tensor("yhalf", [SEG_OUT, W], F16, kind="ExternalOutput")

    from contextlib import ExitStack

    with TileContext(nc) as tc:
        with (
            tc.tile_pool(name="wconst", bufs=1) as cpool,
            tc.tile_pool(name="xp", bufs=NXBUF) as xpool,
            tc.tile_pool(name="op", bufs=4) as opool,
            tc.tile_pool(name="psum", bufs=3, space="PSUM") as pspool,
            ExitStack() as stack,
        ):
            bandt = cpool.tile([KDIM, SIZE * MPAD], F16)
            bandpt = cpool.tile([PK_K, SIZE * MPAD], F16)
            xt = cpool.tile([PK_K, INCOLS], F16)   # fused-tail moving tile
            nc.sync.dma_start(out=bandt[:, :], in_=band[:, :])
            nc.scalar.dma_start(out=bandpt[:, :], in_=bandp[:, :])

            # The x tiles rotate through NXBUF fixed buffers; partition 0 is
            # the all-ones row the constant rides against.  Input DMAs only
            # write partitions 1.., so one memset per buffer at program
            # start suffices (single-partition memsets are slow on DVE).
            for _ in range(NXBUF):
                t = xpool.tile([KDIM, INCOLS], F16, tag="xg")
                nc.vector.memset(t[0:1, :], 1.0)
            nc.vector.memset(xt[0:1, :], 1.0)

            if repeat > 1:
                stack.enter_context(
                    tc.For_i(
                        0, repeat, 1,
                        hint_engines=(
                            mybir.EngineType.PE,
                            mybir.EngineType.DVE,
                            mybir.EngineType.Activation,
                            mybir.EngineType.SP,
                        ),
                        staggered_reset=STAGGERED,
                    )
                )

            def evict_and_store(ps0, ps1, stores):
                # stores: list of (psum row range, dram tensor, dram row0)
                lo = min(r0 for r0, _, _, _ in stores)
                hi = max(r1 for _, r1, _, _ in stores)
                otw = opool.tile([KDIM, W], F16, tag="otw")
                nc.vector.tensor_scalar(
                    otw[lo:hi, 0:NCHUNK], ps0[lo:hi, :], 0.0, 1.0,
                    mybir.AluOpType.max, mybir.AluOpType.min,
                )
                nc.vector.tensor_scalar(
                    otw[lo:hi, NCHUNK:W], ps1[lo:hi, :], 0.0, 1.0,
                    mybir.AluOpType.max, mybir.AluOpType.min,
                )
                for r0, r1, yt, y0 in stores:
                    nc.gpsimd.dma_start(
                        out=yt[y0:y0 + (r1 - r0), :], in_=otw[r0:r1, :])

            def mm_group(wt, kdim, xg, ps0, ps1):
                for j in range(SIZE):
                    nc.tensor.matmul(
                        ps0[:, :],
                        wt[0:kdim, j * MPAD:(j + 1) * MPAD],
                        xg[0:kdim, j:j + NCHUNK],
                        start=(j == 0), stop=(j == SIZE - 1),
                    )
                    nc.tensor.matmul(
                        ps1[:, :],
                        wt[0:kdim, j * MPAD:(j + 1) * MPAD],
                        xg[0:kdim, NCHUNK + j:NCHUNK + j + NCHUNK],
                        start=(j == 0), stop=(j == SIZE - 1),
                    )

            for xin, yout, group_m0 in (
                (xfull, yfull, FULL_M0),
                (xhalf, yhalf, HALF_M0),
            ):
                for m0 in group_m0:
                    nrows = KDIM - 1
                    xg = xpool.tile([KDIM, INCOLS], F16, tag="xg")
                    # bulk of the input on the two HWDGE rings (immune to
                    # DVE/SWDGE SBUF contention), tail on SWDGE
                    q = (2 * nrows) // 5
                    nc.sync.dma_start(
                        out=xg[1:1 + q, :], in_=xin[m0:m0 + q, :])
                    nc.scalar.dma_start(
                        out=xg[1 + q:1 + 2 * q, :],
                        in_=xin[m0 + q:m0 + 2 * q, :])
                    nc.gpsimd.dma_start(
                        out=xg[1 + 2 * q:1 + nrows, :],
                        in_=xin[m0 + 2 * q:m0 + nrows, :])

                    ps0 = pspool.tile([KDIM, NCHUNK], F32, tag="ps0")
                    ps1 = pspool.tile([KDIM, NCHUNK], F32, tag="ps1")
                    mm_group(bandt, KDIM, xg, ps0, ps1)
                    evict_and_store(ps0, ps1, [(0, MG, yout, m0)])

            # fused tail: full-plane rows 984.. and half-plane rows 492..
            # stacked in the contraction dim with a block-diagonal stationary
            nc.sync.dma_start(
                out=xt[1:1 + FT_NR, :], in_=xfull[FT_M0:FT_M0 + FT_NR, :])
            nc.scalar.dma_start(
                out=xt[1 + FT_NR:PK_K, :], in_=xhalf[HT_M0:HT_M0 + HT_NR, :])
            ps0 = pspool.tile([KDIM, NCHUNK], F32, tag="ps0")
            ps1 = pspool.tile([KDIM, NCHUNK], F32, tag="ps1")
            mm_group(bandpt, PK_K, xt, ps0, ps1)
            evict_and_store(ps0, ps1, [
                (0, FT_MG, yfull, FT_M0),
                (FT_MG, FT_MG + HT_MG, yhalf, HT_M0),
            ])
    nc.compile()
    return nc


def _build_weights(wE, bE, wS, bS):
    # match the reference's fp32 arithmetic for the tap values
    a32 = np.einsum("kd,kd->k", wS, wE).astype(np.float32)
    c32 = (np.einsum("kd,kd->k", wS, bE).astype(np.float32)
           + bS.astype(np.float32)).astype(np.float32)
    wp = (a32 / np.float32(SIZE * SIZE)).astype(np.float32).reshape(SIZE, SIZE)
    cprime = np.float32(c32.sum(dtype=np.float32) / np.float32(SIZE * SIZE))

    # split the constant across the 5 bands' row 0 so fp16 storage stays
    # fp32-accurate: c0 = fp16(c'), c1 = fp16(c' - c0), ...
    cparts = np.zeros(SIZE, np.float64)
    rem = np.float64(cprime)
    for j in range(SIZE):
        cj = np.float64(np.float16(rem))
        cparts[j] = cj
        rem -= cj

    band = np.zeros((KDIM, SIZE, MPAD), np.float32)
    for j in range(SIZE):
        band[0, j, :] = cparts[j]
        for i in range(SIZE):
            # out row m uses x row m+i, stored at partition 1+m+i
            mm = np.arange(0, min(MPAD, KDIM - 1 - i))
            band[1 + mm + i, j, mm] = wp[i, j]

    # fused tail: block-diagonal stationary.  Block 1 (full-plane tail):
    # data partitions 1..44, outputs cols 0..39.  Block 2 (half-plane
    # tail): data partitions 45..68, outputs cols 40..59.  Both blocks'
    # constants ride on the shared ones row (partition 0).
    bandp = np.zeros((PK_K, SIZE, MPAD), np.float32)
    b2 = 1 + FT_NR              # 45: block-2 data partition base
    for j in range(SIZE):
        bandp[0, j, 0:FT_MG + HT_MG] = cparts[j]
        for i in range(SIZE):
            mm = np.arange(0, FT_MG)
            bandp[1 + mm + i, j, mm] = wp[i, j]
            mm = np.arange(0, HT_MG)
            bandp[b2 + mm + i, j, FT_MG + mm] = wp[i, j]
    return (band.reshape(KDIM, SIZE * MPAD).astype(np.float16),
            bandp.reshape(PK_K, SIZE * MPAD).astype(np.float16))


def kernel(x, wE, bE, wS, bS, _trace=False):
    x = np.asarray(x, dtype=np.float32)
    planes = np.clip(x, 0.0, 1.0).astype(np.float16).reshape(BS * C, H, W)
    xp = np.pad(planes, ((0, 0), (PAD, PAD), (PAD, PAD)), mode="reflect")

    band, bandp = _build_weights(
        np.asarray(wE, np.float32), np.asarray(bE, np.float32),
        np.asarray(wS, np.float32), np.asarray(bS, np.float32),
    )

    in_maps = []
    for core in range(NCORES):
        hp = 8 + core // 2          # half-plane source: planes 8..11
        half = core % 2
        in_maps.append({
            "xfull": xp[core],
            "xhalf": xp[hp, half * SEG_OUT: half * SEG_OUT + SEG_IN, :],
            "band": band,
            "bandp": bandp,
        })

    key = ("prog", REPEAT, VARIANT, STAGGERED)
    if key not in _prog_cache:
        _prog_cache[key] = _build_program(REPEAT, VARIANT)
    nc = _prog_cache[key]

    res = run_bass_kernel_spmd(
        nc, in_maps, core_ids=list(range(NCORES)), trace=bool(_trace)
    )

    out = np.empty((BS * C, H, W), np.float32)
    for core in range(NCORES):
        out[core] = res.results[core]["yfull"]
        hp = 8 + core // 2
        half = core % 2
        out[hp, half * SEG_OUT:(half + 1) * SEG_OUT, :] = res.results[core]["yhalf"]
    out = out.reshape(BS, C, H, W)

    if _trace:
        return out, res
    return out
